# revision 10
# baseline (speedup 1.0000x reference)
"""Trainium2 Bass kernel for nn_Expert_13082470383822.

y = silu(depthwise_causal_conv1d(x, conv_w, K=4) + conv_b);  out = y @ W_proj.T + b_proj
x [4, 4096, 2048] fp32. Data-parallel over the 16384 (batch*seq) tokens across
8 NeuronCores (2048 tokens/core + 3-token halo).

bf16 datapath. Per-core schedule:
- ~56 dummy N=64 matmuls at kernel start keep the PE busy through the HAM
  activity window so the real matmuls run at 2.4 GHz from the first issue.
- strips 0-2 run the projection j-outer over half-strips (8 PSUM banks =
  2 m-tiles x 4 e-chunks, all 16 contraction tiles per phase): W[j] is
  consumed at ~1.7us/tile which matches the DMA arrival rate, so the PE
  never waits on the W stream; strip 3 is e-outer per m-tile so the final
  copybacks overlap the remaining accumulation.
- conv per j: tap0 via DVE tensor_scalar (2x mode), taps 2,3 via
  scalar_tensor_tensor, tap1 on the ACT engine (Copy with per-partition
  scale), one tensor_tensor combine, SiLU+conv_b on ACT. Strip 0 is
  convolved in 256-token halves so the first matmul issues ~1.5us after the
  first x tile lands (a tiny j0-only tile loaded ahead of everything).
- PSUM->SBUF copybacks at phase boundaries are split ACT/DVE (4+4) so all
  8 banks recycle in ~1.4us; b_proj is added on the host.
"""

import sys

if "/opt/trn_rl_repo" not in sys.path:
    sys.path.insert(0, "/opt/trn_rl_repo")

import os

import numpy as np

if os.environ.get("BASS_LDW_OPT", "0") == "1":
    import concourse.bass_utils as _bu

    if not getattr(_bu, "_ldw_opt_patched", False):
        _orig_run_command = _bu.run_command

        def _run_command_ldw(cmd, *a, **kw):
            cmd = [
                "--enable-ldw-opt=true" if c == "--enable-ldw-opt=false" else c
                for c in cmd
            ]
            return _orig_run_command(cmd, *a, **kw)

        _bu.run_command = _run_command_ldw
        _bu._ldw_opt_patched = True

B, S, D, KW = 4, 4096, 2048, 4
NCORES = 8
T = (B * S) // NCORES  # tokens per core = 2048
KT = D // 128  # 16 channel tiles
ECH = D // 512  # 4 e-chunks of the output features
CW = 512  # conv strip width (tokens)
MS = 128  # matmul stationary strip width (tokens)
NCS = T // CW  # 4 conv strips
MPC = CW // MS  # 4 matmul strips per conv strip
JQ = 4  # j-tiles per x quarter-load
NDUM = 56  # warm-up matmuls

_BUILT = {}


def _build_program():
    if "nc" in _BUILT:
        return _BUILT["nc"]

    import concourse.tile as tile
    from concourse import bacc, mybir

    dt = mybir.dt
    AF = mybir.ActivationFunctionType
    ALU = mybir.AluOpType

    nc = bacc.Bacc("TRN2", target_bir_lowering=False, debug=False)
    # pre-tiled x: per (conv-strip, j-quarter): [128, 4*(CW+3)] bf16 contiguous
    xs_d = nc.declare_dram_parameter(
        "xs_t", [NCS * (KT // JQ), 128, JQ * (CW + 3)], dt.bfloat16, isOutput=False
    )
    # tiny j0-only head of strip 0 (tokens -3..257) to start conv early
    xs0_d = nc.declare_dram_parameter("xs0", [128, 260], dt.bfloat16, isOutput=False)
    wt = nc.declare_dram_parameter("wt", [D, D], dt.bfloat16, isOutput=False)
    cw = nc.declare_dram_parameter("cw", [128, KT * KW], dt.float32, isOutput=False)
    cb = nc.declare_dram_parameter("cb", [128, KT], dt.float32, isOutput=False)
    out = nc.declare_dram_parameter("out", [T, D], dt.bfloat16, isOutput=True)

    with tile.TileContext(nc) as tc:
        with (
            tc.tile_pool(name="consts", bufs=1) as cpool,
            tc.tile_pool(name="wpool", bufs=1) as wpool,
            tc.tile_pool(name="xpool", bufs=12) as xpool,
            tc.tile_pool(name="ypool", bufs=3) as ypool,
            tc.tile_pool(name="apool", bufs=6) as apool,
            tc.tile_pool(name="opool", bufs=8) as opool,
            tc.tile_pool(name="pspool", bufs=8, space="PSUM") as pspool,
        ):
            # ---- warm-up: ACT table + HAM via dummy matmuls ----
            dmw = cpool.tile([128, 64], dt.bfloat16, name="dmw")
            nc.vector.memset(dmw[:, :], 0.0)
            dum = cpool.tile([1, 1], dt.float32, name="dum")
            nc.vector.memset(dum[:, :], 0.0)
            nc.scalar.activation(dum[:, :], dum[:, :], AF.Silu, bias=0.0)
            dps = pspool.tile([128, 512], dt.float32, name="ps", tag="ps")
            for i in range(NDUM):
                nc.tensor.matmul(
                    dps[0:64, 0:64],
                    dmw[:, 0:64],
                    dmw[:, 0:64],
                    start=(i == 0),
                    stop=(i == NDUM - 1),
                )

            # ---- consts ----
            cw_sb = cpool.tile([128, KT * KW], dt.float32, name="cw_sb")
            nc.gpsimd.dma_start(out=cw_sb[:, :], in_=cw[:, :])
            cb_sb = cpool.tile([128, KT], dt.float32, name="cb_sb")
            nc.gpsimd.dma_start(out=cb_sb[:, :], in_=cb[:, :])

            # ---- startup DMA: interleaved by first-use time across the two
            # queues (sync: x0 quarters + odd W; gpsimd: even W then x1) ----
            xs0 = cpool.tile([128, 260], dt.bfloat16, name="xs0")
            nc.sync.dma_start(out=xs0[:, :], in_=xs0_d[:, :])

            xq = {}

            def load_xq(c, q, eng):
                xt = xpool.tile([128, JQ, CW + 3], dt.bfloat16, name="xs", tag="xs")
                eng.dma_start(
                    out=xt[:, :, :],
                    in_=xs_d[c * (KT // JQ) + q, :, :].rearrange(
                        "p (j t) -> p j t", j=JQ
                    ),
                )
                xq[(c, q)] = xt

            w_sb = [None] * KT

            def load_w(j, eng):
                wj = wpool.tile([128, D], dt.bfloat16, name=f"w{j}")
                eng.dma_start(out=wj[:, :], in_=wt[j * 128 : (j + 1) * 128, :])
                w_sb[j] = wj

            # sync (SP HWDGE) carries the startup-critical path (W0/W1, x0)
            # then odd W and the later x strips; the SWDGE gpsimd queue gets
            # only 7 even W tiles (few enough that its ring never blocks the
            # gpsimd TT-combine ops below); ACT stays pure compute.
            load_w(0, nc.sync)
            load_xq(0, 0, nc.sync)
            load_w(2, nc.gpsimd)
            load_w(1, nc.sync)
            load_w(4, nc.gpsimd)
            load_xq(0, 1, nc.sync)
            load_w(6, nc.gpsimd)
            load_w(3, nc.sync)
            load_w(8, nc.gpsimd)
            load_xq(0, 2, nc.sync)
            load_w(10, nc.gpsimd)
            load_w(5, nc.sync)
            load_w(12, nc.gpsimd)
            load_xq(0, 3, nc.sync)
            load_w(14, nc.gpsimd)
            load_w(7, nc.sync)
            load_w(9, nc.sync)
            load_w(11, nc.sync)
            load_w(13, nc.sync)
            load_w(15, nc.sync)
            for q in range(4):
                load_xq(1, q, nc.sync)
            for q in range(4):
                load_xq(2, q, nc.sync)
            for q in range(4):
                load_xq(3, q, nc.sync)

            # ---- conv emission helpers ----
            ys_strip = {}

            def conv_j(c, j, lo, width, pure_dve=False, src=None):
                """Emit conv for j over out tokens [lo, lo+width) of strip c."""
                xs = src if src is not None else xq[(c, j // JQ)]
                jj = j % JQ

                def tap(k):
                    if src is not None:
                        return xs[:, lo + k : lo + k + width]
                    return xs[:, jj, lo + k : lo + k + width]

                ys = ys_strip[c]
                t1 = apool.tile([128, 512], dt.bfloat16, name="t1", tag="t1")
                if pure_dve:
                    nc.vector.tensor_scalar(
                        t1[:, 0:width], tap(0), cw_sb[:, j * KW : j * KW + 1],
                        None, ALU.mult,
                    )
                    for k in range(1, KW):
                        nc.vector.scalar_tensor_tensor(
                            t1[:, 0:width], tap(k),
                            cw_sb[:, j * KW + k : j * KW + k + 1],
                            t1[:, 0:width], ALU.mult, ALU.add,
                        )
                    nc.scalar.activation(
                        ys[:, j, lo : lo + width], t1[:, 0:width], AF.Silu,
                        bias=cb_sb[:, j : j + 1],
                    )
                    return
                # ACT computes tap1 in parallel with the DVE chain
                t2 = apool.tile([128, 512], dt.bfloat16, name="t2", tag="t2")
                nc.scalar.activation(
                    t2[:, 0:width], tap(1), AF.Copy,
                    scale=cw_sb[:, j * KW + 1 : j * KW + 2],
                )
                nc.vector.tensor_scalar(
                    t1[:, 0:width], tap(0), cw_sb[:, j * KW : j * KW + 1],
                    None, ALU.mult,
                )
                for k in (2, 3):
                    nc.vector.scalar_tensor_tensor(
                        t1[:, 0:width], tap(k),
                        cw_sb[:, j * KW + k : j * KW + k + 1],
                        t1[:, 0:width], ALU.mult, ALU.add,
                    )
                t3 = apool.tile([128, 512], dt.bfloat16, name="t3", tag="t3")
                # combine on the (otherwise idle) gpsimd engine
                nc.gpsimd.tensor_tensor(
                    t3[:, 0:width], t1[:, 0:width], t2[:, 0:width], ALU.add
                )
                nc.scalar.activation(
                    ys[:, j, lo : lo + width], t3[:, 0:width], AF.Silu,
                    bias=cb_sb[:, j : j + 1],
                )

            # ---- matmul phase helpers (j-outer over half-strip) ----
            def mm_phase(c, half):
                ys = ys_strip[c]
                pss = [
                    pspool.tile([128, 512], dt.float32, name="ps", tag="ps")
                    for _ in range(8)
                ]
                for j in range(KT):
                    for ml in range(2):
                        m = 2 * half + ml
                        for e in range(ECH):
                            nc.tensor.matmul(
                                pss[ml * ECH + e][:, :],
                                ys[:, j, m * MS : (m + 1) * MS],
                                w_sb[j][:, e * 512 : (e + 1) * 512],
                                start=(j == 0),
                                stop=(j == KT - 1),
                            )
                return pss

            def copyback(c, half, pss, bank_eng=None):
                """Copy 8 banks to SBUF (ACT/DVE alternating) + store."""
                for b in range(8):
                    ml, e = b // ECH, b % ECH
                    m = 2 * half + ml
                    s = c * MPC + m
                    os_sb = opool.tile([128, 512], dt.bfloat16, name="os", tag="os")
                    if b % 2 == 0:
                        nc.scalar.copy(os_sb[:, :], pss[b][:, :])
                    else:
                        nc.vector.tensor_copy(os_sb[:, :], pss[b][:, :])
                    nc.sync.dma_start(
                        out=out[s * MS : (s + 1) * MS, e * 512 : (e + 1) * 512],
                        in_=os_sb[:, :],
                    )

            # ================= schedule =================
            # strip 0 conv h0 (halves), j0 from the tiny head tile
            ys_strip[0] = ypool.tile([128, KT, CW], dt.bfloat16, name="ys", tag="ys")
            conv_j(0, 0, 0, 256, pure_dve=True, src=xs0)
            for j in range(1, KT):
                conv_j(0, j, 0, 256)

            pss_a = mm_phase(0, 0)

            # strip 0 conv h1: a few j, then boundary copies, then the rest
            for j in range(0, 5):
                conv_j(0, j, 256, 256)
            copyback(0, 0, pss_a)
            for j in range(5, KT):
                conv_j(0, j, 256, 256)

            pss_b = mm_phase(0, 1)

            # strip 1 conv (full width)
            ys_strip[1] = ypool.tile([128, KT, CW], dt.bfloat16, name="ys", tag="ys")
            for j in range(0, 2):
                conv_j(1, j, 0, 512)
            copyback(0, 1, pss_b)
            for j in range(2, KT):
                conv_j(1, j, 0, 512)

            pss_c = mm_phase(1, 0)

            ys_strip[2] = ypool.tile([128, KT, CW], dt.bfloat16, name="ys", tag="ys")
            conv_j(2, 0, 0, 512)
            copyback(1, 0, pss_c)

            pss_d = mm_phase(1, 1)

            for j in range(1, KT):
                conv_j(2, j, 0, 512)
            copyback(1, 1, pss_d)

            pss_e = mm_phase(2, 0)

            ys_strip[3] = ypool.tile([128, KT, CW], dt.bfloat16, name="ys", tag="ys")
            for j in range(0, 11):
                conv_j(3, j, 0, 512)
            copyback(2, 0, pss_e)

            pss_f = mm_phase(2, 1)

            for j in range(11, KT):
                conv_j(3, j, 0, 512)
            copyback(2, 1, pss_f)

            # strip 3: e-outer per m (copybacks overlap next accumulation)
            ys = ys_strip[3]
            for m in range(MPC):
                s = 3 * MPC + m
                pss = []
                for e in range(ECH):
                    ps = pspool.tile([128, 512], dt.float32, name="ps", tag="ps")
                    for j in range(KT):
                        nc.tensor.matmul(
                            ps[:, :],
                            ys[:, j, m * MS : (m + 1) * MS],
                            w_sb[j][:, e * 512 : (e + 1) * 512],
                            start=(j == 0),
                            stop=(j == KT - 1),
                        )
                    pss.append(ps)
                for e in range(ECH):
                    os_sb = opool.tile([128, 512], dt.bfloat16, name="os", tag="os")
                    if m == MPC - 1 and e == ECH - 1:
                        # final tile: split the copy across ACT+DVE to shave
                        # the drain tail
                        nc.scalar.copy(os_sb[:, 0:256], pss[e][:, 0:256])
                        nc.vector.tensor_copy(os_sb[:, 256:512], pss[e][:, 256:512])
                    elif e % 2 == 0:
                        nc.scalar.copy(os_sb[:, :], pss[e][:, :])
                    else:
                        nc.vector.tensor_copy(os_sb[:, :], pss[e][:, :])
                    nc.sync.dma_start(
                        out=out[s * MS : (s + 1) * MS, e * 512 : (e + 1) * 512],
                        in_=os_sb[:, :],
                    )

    nc.compile()
    _BUILT["nc"] = nc
    return nc


def _shard_inputs(x, conv_w, conv_b, W_proj, b_proj):
    import ml_dtypes

    bf16 = ml_dtypes.bfloat16
    wt_np = np.ascontiguousarray(W_proj.T.astype(bf16))
    cw_np = np.ascontiguousarray(
        conv_w.reshape(KT, 128, KW).transpose(1, 0, 2).reshape(128, KT * KW),
        dtype=np.float32,
    )
    cb_np = np.ascontiguousarray(conv_b.reshape(KT, 128).T, dtype=np.float32)

    x16 = x.astype(bf16)
    per_batch = S // T
    in_maps = []
    for c in range(NCORES):
        b = c // per_batch
        s0 = (c % per_batch) * T
        xp = np.zeros((T + 3, D), dtype=bf16)
        xp[3:] = x16[b, s0 : s0 + T]
        if s0 > 0:
            xp[:3] = x16[b, s0 - 3 : s0]
        xTc = xp.T  # [D, T+3]
        # [NCS, D, CW+3] sliding strips -> [NCS, 16, 128, CW+3]
        strips = np.stack([xTc[:, i * CW : i * CW + CW + 3] for i in range(NCS)])
        strips = strips.reshape(NCS, KT, 128, CW + 3)
        # -> [NCS, 4 quarters, 128, 4*(CW+3)]
        quarters = np.ascontiguousarray(
            strips.reshape(NCS, KT // JQ, JQ, 128, CW + 3).transpose(0, 1, 3, 2, 4)
        ).reshape(NCS * (KT // JQ), 128, JQ * (CW + 3))
        xs0_np = np.ascontiguousarray(xTc[0:128, 0:260])
        in_maps.append(
            {
                "xs_t": quarters,
                "xs0": xs0_np,
                "wt": wt_np,
                "cw": cw_np,
                "cb": cb_np,
            }
        )
    return in_maps


def run_sharded(x, conv_w, conv_b, W_proj, b_proj, trace=False):
    """Run across the 8 cores; returns (full_out [B,S,D], BassKernelResults)."""
    from concourse.bass_utils import run_bass_kernel_spmd

    nc = _build_program()
    in_maps = _shard_inputs(x, conv_w, conv_b, W_proj, b_proj)
    try:
        res = run_bass_kernel_spmd(nc, in_maps, list(range(NCORES)), trace=trace)
    except Exception:
        # transient device wedges (NRT_EXEC_UNIT_UNRECOVERABLE) clear on retry
        res = run_bass_kernel_spmd(nc, in_maps, list(range(NCORES)), trace=trace)
    full = np.empty((B, S, D), dtype=np.float32)
    per_batch = S // T
    bp = b_proj.astype(np.float32)
    for c in range(NCORES):
        b = c // per_batch
        s0 = (c % per_batch) * T
        full[b, s0 : s0 + T] = res.results[c]["out"].astype(np.float32) + bp
    return full, res


def kernel(x, conv_w, conv_b, W_proj, b_proj):
    full, _ = run_sharded(x, conv_w, conv_b, W_proj, b_proj, trace=False)
    return full


# revision 11
# speedup vs baseline: 1.2037x; 1.2037x over previous
"""Trainium2 Bass kernel for nn_Expert_13082470383822.

y = silu(depthwise_causal_conv1d(x, conv_w, K=4) + conv_b);  out = y @ W_proj.T + b_proj
x [4, 4096, 2048] fp32. Data-parallel over the 16384 (batch*seq) tokens across
8 NeuronCores (2048 tokens/core + 3-token halo).

bf16 datapath. Per-core schedule:
- ~56 dummy N=64 matmuls at kernel start keep the PE busy through the HAM
  activity window so the real matmuls run at 2.4 GHz from the first issue.
- strips 0-2 run the projection j-outer over half-strips (8 PSUM banks =
  2 m-tiles x 4 e-chunks, all 16 contraction tiles per phase): W[j] is
  consumed at ~1.7us/tile which matches the DMA arrival rate, so the PE
  never waits on the W stream; strip 3 is e-outer per m-tile so the final
  copybacks overlap the remaining accumulation.
- conv per j: tap0 via DVE tensor_scalar (2x mode), taps 2,3 via
  scalar_tensor_tensor, tap1 on the ACT engine (Copy with per-partition
  scale), one tensor_tensor combine, SiLU+conv_b on ACT. Strip 0 is
  convolved in 256-token halves so the first matmul issues ~1.5us after the
  first x tile lands (a tiny j0-only tile loaded ahead of everything).
- PSUM->SBUF copybacks at phase boundaries are split ACT/DVE (4+4) so all
  8 banks recycle in ~1.4us; b_proj is added on the host.
"""

import sys

if "/opt/trn_rl_repo" not in sys.path:
    sys.path.insert(0, "/opt/trn_rl_repo")

import os

import numpy as np

if os.environ.get("BASS_LDW_OPT", "0") == "1":
    import concourse.bass_utils as _bu

    if not getattr(_bu, "_ldw_opt_patched", False):
        _orig_run_command = _bu.run_command

        def _run_command_ldw(cmd, *a, **kw):
            cmd = [
                "--enable-ldw-opt=true" if c == "--enable-ldw-opt=false" else c
                for c in cmd
            ]
            return _orig_run_command(cmd, *a, **kw)

        _bu.run_command = _run_command_ldw
        _bu._ldw_opt_patched = True

B, S, D, KW = 4, 4096, 2048, 4
NCORES = 8
T = (B * S) // NCORES  # tokens per core = 2048
KT = D // 128  # 16 channel tiles
ECH = D // 512  # 4 e-chunks of the output features
CW = 512  # conv strip width (tokens)
MS = 128  # matmul stationary strip width (tokens)
NCS = T // CW  # 4 conv strips
MPC = CW // MS  # 4 matmul strips per conv strip
JQ = 4  # j-tiles per x quarter-load
NDUM = 56  # warm-up matmuls

_BUILT = {}


def _build_program():
    if "nc" in _BUILT:
        return _BUILT["nc"]

    import concourse.tile as tile
    from concourse import bacc, mybir

    dt = mybir.dt
    AF = mybir.ActivationFunctionType
    ALU = mybir.AluOpType

    nc = bacc.Bacc("TRN2", target_bir_lowering=False, debug=False)
    # pre-tiled x: per (conv-strip, j-quarter): [128, 4*(CW+3)] bf16 contiguous
    xs_d = nc.declare_dram_parameter(
        "xs_t", [NCS * (KT // JQ), 128, JQ * (CW + 3)], dt.bfloat16, isOutput=False
    )
    # tiny j0-only head of strip 0 (tokens -3..257) to start conv early
    xs0_d = nc.declare_dram_parameter("xs0", [128, 260], dt.bfloat16, isOutput=False)
    wt = nc.declare_dram_parameter("wt", [D, D], dt.bfloat16, isOutput=False)
    cw = nc.declare_dram_parameter("cw", [128, KT * KW], dt.float32, isOutput=False)
    cb = nc.declare_dram_parameter("cb", [128, KT], dt.float32, isOutput=False)
    out = nc.declare_dram_parameter("out", [T, D], dt.bfloat16, isOutput=True)

    with tile.TileContext(nc) as tc:
        with (
            tc.tile_pool(name="consts", bufs=1) as cpool,
            tc.tile_pool(name="wpool", bufs=1) as wpool,
            tc.tile_pool(name="xpool", bufs=12) as xpool,
            tc.tile_pool(name="ypool", bufs=3) as ypool,
            tc.tile_pool(name="apool", bufs=6) as apool,
            tc.tile_pool(name="opool", bufs=8) as opool,
            tc.tile_pool(name="pspool", bufs=8, space="PSUM") as pspool,
        ):
            # ---- warm-up: ACT table + HAM via dummy matmuls ----
            dmw = cpool.tile([128, 64], dt.bfloat16, name="dmw")
            nc.vector.memset(dmw[:, :], 0.0)
            dum = cpool.tile([1, 1], dt.float32, name="dum")
            nc.vector.memset(dum[:, :], 0.0)
            nc.scalar.activation(dum[:, :], dum[:, :], AF.Silu, bias=0.0)
            dps = pspool.tile([128, 512], dt.float32, name="ps", tag="ps")
            for i in range(NDUM):
                nc.tensor.matmul(
                    dps[0:64, 0:64],
                    dmw[:, 0:64],
                    dmw[:, 0:64],
                    start=(i == 0),
                    stop=(i == NDUM - 1),
                )

            # ---- consts ----
            cw_sb = cpool.tile([128, KT * KW], dt.float32, name="cw_sb")
            nc.gpsimd.dma_start(out=cw_sb[:, :], in_=cw[:, :])
            cb_sb = cpool.tile([128, KT], dt.float32, name="cb_sb")
            nc.gpsimd.dma_start(out=cb_sb[:, :], in_=cb[:, :])

            # ---- startup DMA: interleaved by first-use time across the two
            # queues (sync: x0 quarters + odd W; gpsimd: even W then x1) ----
            xs0 = cpool.tile([128, 260], dt.bfloat16, name="xs0")
            nc.sync.dma_start(out=xs0[:, :], in_=xs0_d[:, :])

            xq = {}

            def load_xq(c, q, eng):
                xt = xpool.tile([128, JQ, CW + 3], dt.bfloat16, name="xs", tag="xs")
                eng.dma_start(
                    out=xt[:, :, :],
                    in_=xs_d[c * (KT // JQ) + q, :, :].rearrange(
                        "p (j t) -> p j t", j=JQ
                    ),
                )
                xq[(c, q)] = xt

            w_sb = [None] * KT

            def load_w(j, eng):
                wj = wpool.tile([128, D], dt.bfloat16, name=f"w{j}")
                eng.dma_start(out=wj[:, :], in_=wt[j * 128 : (j + 1) * 128, :])
                w_sb[j] = wj

            # sync (SP HWDGE) carries the startup-critical path (W0/W1, x0)
            # then odd W and the later x strips; the SWDGE gpsimd queue gets
            # only 7 even W tiles (few enough that its ring never blocks the
            # gpsimd TT-combine ops below); ACT stays pure compute.
            load_w(0, nc.sync)
            load_xq(0, 0, nc.sync)
            load_w(2, nc.gpsimd)
            load_w(1, nc.sync)
            load_w(4, nc.gpsimd)
            load_xq(0, 1, nc.sync)
            load_w(6, nc.gpsimd)
            load_w(3, nc.sync)
            load_w(8, nc.gpsimd)
            load_xq(0, 2, nc.sync)
            load_w(10, nc.gpsimd)
            load_w(5, nc.sync)
            load_w(12, nc.gpsimd)
            load_xq(0, 3, nc.sync)
            load_w(14, nc.gpsimd)
            load_w(7, nc.sync)
            load_w(9, nc.sync)
            load_w(11, nc.sync)
            load_w(13, nc.sync)
            load_w(15, nc.sync)
            for q in range(4):
                load_xq(1, q, nc.sync)
            for q in range(4):
                load_xq(2, q, nc.sync)
            for q in range(4):
                load_xq(3, q, nc.sync)

            # ---- conv emission helpers ----
            ys_strip = {}

            def conv_j(c, j, lo, width, pure_dve=False, src=None):
                """Emit conv for j over out tokens [lo, lo+width) of strip c."""
                xs = src if src is not None else xq[(c, j // JQ)]
                jj = j % JQ

                def tap(k):
                    if src is not None:
                        return xs[:, lo + k : lo + k + width]
                    return xs[:, jj, lo + k : lo + k + width]

                ys = ys_strip[c]
                t1 = apool.tile([128, 512], dt.bfloat16, name="t1", tag="t1")
                if pure_dve:
                    nc.vector.tensor_scalar(
                        t1[:, 0:width], tap(0), cw_sb[:, j * KW : j * KW + 1],
                        None, ALU.mult,
                    )
                    for k in range(1, KW):
                        nc.vector.scalar_tensor_tensor(
                            t1[:, 0:width], tap(k),
                            cw_sb[:, j * KW + k : j * KW + k + 1],
                            t1[:, 0:width], ALU.mult, ALU.add,
                        )
                    nc.scalar.activation(
                        ys[:, j, lo : lo + width], t1[:, 0:width], AF.Silu,
                        bias=cb_sb[:, j : j + 1],
                    )
                    return
                # ACT computes tap1 in parallel with the DVE chain
                t2 = apool.tile([128, 512], dt.bfloat16, name="t2", tag="t2")
                nc.scalar.activation(
                    t2[:, 0:width], tap(1), AF.Copy,
                    scale=cw_sb[:, j * KW + 1 : j * KW + 2],
                )
                nc.vector.tensor_scalar(
                    t1[:, 0:width], tap(0), cw_sb[:, j * KW : j * KW + 1],
                    None, ALU.mult,
                )
                for k in (2, 3):
                    nc.vector.scalar_tensor_tensor(
                        t1[:, 0:width], tap(k),
                        cw_sb[:, j * KW + k : j * KW + k + 1],
                        t1[:, 0:width], ALU.mult, ALU.add,
                    )
                t3 = apool.tile([128, 512], dt.bfloat16, name="t3", tag="t3")
                nc.vector.tensor_tensor(
                    t3[:, 0:width], t1[:, 0:width], t2[:, 0:width], ALU.add
                )
                nc.scalar.activation(
                    ys[:, j, lo : lo + width], t3[:, 0:width], AF.Silu,
                    bias=cb_sb[:, j : j + 1],
                )

            # ---- matmul phase helpers (j-outer over half-strip) ----
            def mm_phase(c, half):
                ys = ys_strip[c]
                pss = [
                    pspool.tile([128, 512], dt.float32, name="ps", tag="ps")
                    for _ in range(8)
                ]
                for j in range(KT):
                    for ml in range(2):
                        m = 2 * half + ml
                        for e in range(ECH):
                            nc.tensor.matmul(
                                pss[ml * ECH + e][:, :],
                                ys[:, j, m * MS : (m + 1) * MS],
                                w_sb[j][:, e * 512 : (e + 1) * 512],
                                start=(j == 0),
                                stop=(j == KT - 1),
                            )
                return pss

            def copyback(c, half, pss, bank_eng=None):
                """Copy 8 banks to SBUF (ACT/DVE alternating) + store."""
                for b in range(8):
                    ml, e = b // ECH, b % ECH
                    m = 2 * half + ml
                    s = c * MPC + m
                    os_sb = opool.tile([128, 512], dt.bfloat16, name="os", tag="os")
                    if b % 2 == 0:
                        nc.scalar.copy(os_sb[:, :], pss[b][:, :])
                    else:
                        nc.vector.tensor_copy(os_sb[:, :], pss[b][:, :])
                    nc.sync.dma_start(
                        out=out[s * MS : (s + 1) * MS, e * 512 : (e + 1) * 512],
                        in_=os_sb[:, :],
                    )

            # ================= schedule =================
            # strip 0 conv h0 (halves), j0 from the tiny head tile
            ys_strip[0] = ypool.tile([128, KT, CW], dt.bfloat16, name="ys", tag="ys")
            conv_j(0, 0, 0, 256, pure_dve=True, src=xs0)
            for j in range(1, KT):
                conv_j(0, j, 0, 256)

            pss_a = mm_phase(0, 0)

            # strip 0 conv h1: a few j, then boundary copies, then the rest
            for j in range(0, 5):
                conv_j(0, j, 256, 256)
            copyback(0, 0, pss_a)
            for j in range(5, KT):
                conv_j(0, j, 256, 256)

            pss_b = mm_phase(0, 1)

            # strip 1 conv (full width)
            ys_strip[1] = ypool.tile([128, KT, CW], dt.bfloat16, name="ys", tag="ys")
            for j in range(0, 2):
                conv_j(1, j, 0, 512)
            copyback(0, 1, pss_b)
            for j in range(2, KT):
                conv_j(1, j, 0, 512)

            pss_c = mm_phase(1, 0)

            ys_strip[2] = ypool.tile([128, KT, CW], dt.bfloat16, name="ys", tag="ys")
            conv_j(2, 0, 0, 512)
            copyback(1, 0, pss_c)

            pss_d = mm_phase(1, 1)

            for j in range(1, KT):
                conv_j(2, j, 0, 512)
            copyback(1, 1, pss_d)

            pss_e = mm_phase(2, 0)

            ys_strip[3] = ypool.tile([128, KT, CW], dt.bfloat16, name="ys", tag="ys")
            for j in range(0, 11):
                conv_j(3, j, 0, 512)
            copyback(2, 0, pss_e)

            pss_f = mm_phase(2, 1)

            for j in range(11, KT):
                conv_j(3, j, 0, 512)
            copyback(2, 1, pss_f)

            # strip 3: e-outer per m (copybacks overlap next accumulation)
            ys = ys_strip[3]
            for m in range(MPC):
                s = 3 * MPC + m
                pss = []
                for e in range(ECH):
                    ps = pspool.tile([128, 512], dt.float32, name="ps", tag="ps")
                    for j in range(KT):
                        nc.tensor.matmul(
                            ps[:, :],
                            ys[:, j, m * MS : (m + 1) * MS],
                            w_sb[j][:, e * 512 : (e + 1) * 512],
                            start=(j == 0),
                            stop=(j == KT - 1),
                        )
                    pss.append(ps)
                for e in range(ECH):
                    os_sb = opool.tile([128, 512], dt.bfloat16, name="os", tag="os")
                    if m == MPC - 1 and e == ECH - 1:
                        # final tile: split the copy across ACT+DVE to shave
                        # the drain tail
                        nc.scalar.copy(os_sb[:, 0:256], pss[e][:, 0:256])
                        nc.vector.tensor_copy(os_sb[:, 256:512], pss[e][:, 256:512])
                    elif e % 2 == 0:
                        nc.scalar.copy(os_sb[:, :], pss[e][:, :])
                    else:
                        nc.vector.tensor_copy(os_sb[:, :], pss[e][:, :])
                    nc.sync.dma_start(
                        out=out[s * MS : (s + 1) * MS, e * 512 : (e + 1) * 512],
                        in_=os_sb[:, :],
                    )

    nc.compile()
    _BUILT["nc"] = nc
    return nc


def _shard_inputs(x, conv_w, conv_b, W_proj, b_proj):
    import ml_dtypes

    bf16 = ml_dtypes.bfloat16
    wt_np = np.ascontiguousarray(W_proj.T.astype(bf16))
    cw_np = np.ascontiguousarray(
        conv_w.reshape(KT, 128, KW).transpose(1, 0, 2).reshape(128, KT * KW),
        dtype=np.float32,
    )
    cb_np = np.ascontiguousarray(conv_b.reshape(KT, 128).T, dtype=np.float32)

    x16 = x.astype(bf16)
    per_batch = S // T
    in_maps = []
    for c in range(NCORES):
        b = c // per_batch
        s0 = (c % per_batch) * T
        xp = np.zeros((T + 3, D), dtype=bf16)
        xp[3:] = x16[b, s0 : s0 + T]
        if s0 > 0:
            xp[:3] = x16[b, s0 - 3 : s0]
        xTc = xp.T  # [D, T+3]
        # [NCS, D, CW+3] sliding strips -> [NCS, 16, 128, CW+3]
        strips = np.stack([xTc[:, i * CW : i * CW + CW + 3] for i in range(NCS)])
        strips = strips.reshape(NCS, KT, 128, CW + 3)
        # -> [NCS, 4 quarters, 128, 4*(CW+3)]
        quarters = np.ascontiguousarray(
            strips.reshape(NCS, KT // JQ, JQ, 128, CW + 3).transpose(0, 1, 3, 2, 4)
        ).reshape(NCS * (KT // JQ), 128, JQ * (CW + 3))
        xs0_np = np.ascontiguousarray(xTc[0:128, 0:260])
        in_maps.append(
            {
                "xs_t": quarters,
                "xs0": xs0_np,
                "wt": wt_np,
                "cw": cw_np,
                "cb": cb_np,
            }
        )
    return in_maps


def run_sharded(x, conv_w, conv_b, W_proj, b_proj, trace=False):
    """Run across the 8 cores; returns (full_out [B,S,D], BassKernelResults)."""
    from concourse.bass_utils import run_bass_kernel_spmd

    nc = _build_program()
    in_maps = _shard_inputs(x, conv_w, conv_b, W_proj, b_proj)
    try:
        res = run_bass_kernel_spmd(nc, in_maps, list(range(NCORES)), trace=trace)
    except Exception:
        # transient device wedges (NRT_EXEC_UNIT_UNRECOVERABLE) clear on retry
        res = run_bass_kernel_spmd(nc, in_maps, list(range(NCORES)), trace=trace)
    full = np.empty((B, S, D), dtype=np.float32)
    per_batch = S // T
    bp = b_proj.astype(np.float32)
    for c in range(NCORES):
        b = c // per_batch
        s0 = (c % per_batch) * T
        full[b, s0 : s0 + T] = res.results[c]["out"].astype(np.float32) + bp
    return full, res


def kernel(x, conv_w, conv_b, W_proj, b_proj):
    full, _ = run_sharded(x, conv_w, conv_b, W_proj, b_proj, trace=False)
    return full
